# revision 14
# baseline (speedup 1.0000x reference)
"""Trainium2 Bass kernel for CorrelatedGraphConv (fp16 redesign).

Reference (per batch, N=100 rows, D=1024, L=2000 labels):
    adj   = (graph != 0)
    lin   = x + x@W0.T + x@W1.T + sum_j bias[graph[:, j]]
    a     = x@Wa.T + ba ; b = x@Wb.T + bb
    alpha = relu(a @ b.T)
    alpha = softmax(adj @ alpha, axis=0)     # over rows i
    out   = alpha @ lin

Design (data-parallel, 2 batches/core):
  * QK rewrite: a@b.T = x (Wa.T Wb) x.T + r 1^T + 1 c'^T with
    M2 = Wb.T@Wa (= M^T), r = x@(Wa.T@bb), c' = x@(Wb.T@ba) + ba.bb
    precomputed on host; the rank-1 terms ride an augmented K=2 matmul.
  * wct = W0.T + W1.T + I folds the "+x" into the linear matmul.
  * x^T, adj^T, identity, doubled-g arrays are host-prepared, killing
    all x/g transposes on device.
  * Label histogram: count[i,j] = #(g[i,:]==g[i,j]) via int16 shifted
    compares on DVE (contiguous APs -> 2x mode; even/odd shift split
    keeps 4B alignment), then gpsimd local_scatter writes count at
    idx=g for EVERY token - duplicate indices race benignly because
    all duplicates carry the same value (HW-verified).
  * Everything fp16 (numpy-emulated rel err 9.7e-4 vs 2e-2 budget).
"""

import numpy as np

import concourse.bass as bass
import concourse.mybir as mybir
import concourse.tile as tile
from concourse import bacc, library_config

F32 = mybir.dt.float32
F16 = mybir.dt.float16
I16 = mybir.dt.int16

B, N, D, L = 16, 100, 1024, 2000
NCORES = 8
BPC = B // NCORES          # 2 batches per core
R = BPC * N                # 200 rows per core
DT = D // 128              # 8 d-tiles
LT = (L + 127) // 128      # 16 label tiles (last is 80 rows)
NCH = 112                  # scatter channels (>=100, mult of 16)
G2W = 2 * N + 2            # doubled g row + pad

_CACHE = {}

ACT = mybir.ActivationFunctionType
ALU = mybir.AluOpType


def _ap3(sl, mid_step, mid_cnt, inner_cnt):
    """[P, F] contiguous slice -> [P, mid, inner] view with raw steps."""
    (pstep, pcount), (fstep, fcount) = sl.ap[0], sl.ap[1]
    assert fstep == 1
    return bass.AP(tensor=sl.tensor, offset=sl.offset,
                   ap=[[pstep, pcount], [mid_step, mid_cnt],
                       [1, inner_cnt]])


def _build_program():
    nc = bacc.Bacc("TRN2", target_bir_lowering=False, debug=False,
                   num_devices=NCORES)
    g_d = nc.declare_dram_parameter("gidx", [NCH, R], I16, isOutput=False)
    g2a_d = nc.declare_dram_parameter("g2a", [N, BPC * G2W], I16, isOutput=False)
    g2b_d = nc.declare_dram_parameter("g2b", [N, BPC * G2W], I16, isOutput=False)
    id16_d = nc.declare_dram_parameter("id16", [128, 128], F16, isOutput=False)
    xt_d = nc.declare_dram_parameter("xt", [128, DT * R], F16, isOutput=False)
    m2_d = nc.declare_dram_parameter("m2", [128, DT * D], F16, isOutput=False)
    auglhs_d = nc.declare_dram_parameter("auglhs", [2, R], F16, isOutput=False)
    augrhs_d = nc.declare_dram_parameter("augrhs", [2, R], F16, isOutput=False)
    adjt_d = nc.declare_dram_parameter("adjt", [N, R], F16, isOutput=False)
    wct_d = nc.declare_dram_parameter("wct", [128, DT * D], F16, isOutput=False)
    bias_d = nc.declare_dram_parameter("biasr", [128, LT * D], F16, isOutput=False)
    out_d = nc.declare_dram_parameter("out", [R, D], F32, isOutput=True)

    with tile.TileContext(nc) as tc:
        _emit(tc, g_d, g2a_d, g2b_d, id16_d, xt_d, m2_d, auglhs_d,
              augrhs_d, adjt_d, wct_d, bias_d, out_d)
    nc.compile()
    return nc


def _emit(tc, g_d, g2a_d, g2b_d, id16_d, xt_d, m2_d, auglhs_d, augrhs_d,
          adjt_d, wct_d, bias_d, out_d):
    nc = tc.nc
    import contextlib

    ctx = contextlib.ExitStack()
    with ctx:
        const = ctx.enter_context(tc.tile_pool(name="const", bufs=1))
        meqp = ctx.enter_context(tc.tile_pool(name="meq", bufs=2))
        cbuf = ctx.enter_context(tc.tile_pool(name="cbuf", bufs=4))
        cmp_ = ctx.enter_context(tc.tile_pool(name="cmat", bufs=2))
        ctp = ctx.enter_context(tc.tile_pool(name="ctm", bufs=2))
        small = ctx.enter_context(tc.tile_pool(name="small", bufs=8))
        linp = ctx.enter_context(tc.tile_pool(name="lin", bufs=2))
        outp = ctx.enter_context(tc.tile_pool(name="outs", bufs=2))
        psA = ctx.enter_context(tc.tile_pool(name="psA", bufs=2, space="PSUM"))
        psB = ctx.enter_context(tc.tile_pool(name="psB", bufs=2, space="PSUM"))
        pslin = ctx.enter_context(tc.tile_pool(name="pslin", bufs=2, space="PSUM"))

        nc.gpsimd.load_library(library_config.local_scatter)

        # ---- input DMAs, ordered by first use ----
        g2a = const.tile([N, BPC * G2W], I16)
        nc.sync.dma_start(out=g2a[:], in_=g2a_d.ap())
        g_sb = const.tile([NCH, R], I16)
        nc.sync.dma_start(out=g_sb[:], in_=g_d.ap())
        g2b = const.tile([N, BPC * G2W], I16)
        nc.sync.dma_start(out=g2b[:], in_=g2b_d.ap())
        xt_sb = const.tile([128, DT * R], F16)
        nc.sync.dma_start(out=xt_sb[:], in_=xt_d.ap())
        m2_sb = const.tile([128, DT * D], F16)
        for mt in range(DT):
            nc.sync.dma_start(out=m2_sb[:, mt * D:(mt + 1) * D],
                              in_=m2_d.ap()[:, mt * D:(mt + 1) * D])
        auglhs = const.tile([2, R], F16)
        nc.sync.dma_start(out=auglhs[:], in_=auglhs_d.ap())
        augrhs = const.tile([2, R], F16)
        nc.sync.dma_start(out=augrhs[:], in_=augrhs_d.ap())
        adjt_sb = const.tile([N, R], F16)
        nc.sync.dma_start(out=adjt_sb[:], in_=adjt_d.ap())
        id16 = const.tile([128, 128], F16)
        nc.sync.dma_start(out=id16[:], in_=id16_d.ap())
        wct_sb = const.tile([128, DT * D], F16)
        for dk in range(DT):
            nc.sync.dma_start(out=wct_sb[:, dk * D:(dk + 1) * D],
                              in_=wct_d.ap()[:, dk * D:(dk + 1) * D])
        bias_sb = const.tile([128, LT * D], F16)
        for lc in range(LT):
            nc.sync.dma_start(out=bias_sb[:, lc * D:(lc + 1) * D],
                              in_=bias_d.ap()[:, lc * D:(lc + 1) * D])

        # ---- DVE dedup chain + scatter; batch 1 first (its cmat gates
        # the PE tail the longest) ----
        cmats = {}
        for b in (1, 0):
            gsl = g_sb[:N, b * N:(b + 1) * N]
            cnt16 = cbuf.tile([NCH, N], F16, tag=f"cnt{b}")

            meqE = meqp.tile([N, 50 * N], I16, tag="meqE")
            meqO = meqp.tile([N, 50 * N], I16, tag="meqO")
            # meqE[i,t,j] = (g[i,j] == g2[i, 2t + j]);  meqO: 2t+1 + j
            nc.vector.tensor_tensor(
                out=_ap3(meqE[:], N, 50, N),
                in0=_ap3(gsl, 0, 50, N),
                in1=_ap3(g2a[:, b * G2W:b * G2W + 2 * N], 2, 50, N),
                op=ALU.is_equal)
            nc.vector.tensor_tensor(
                out=_ap3(meqO[:], N, 50, N),
                in0=_ap3(gsl, 0, 50, N),
                in1=_ap3(g2b[:, b * G2W:b * G2W + 2 * N], 2, 50, N),
                op=ALU.is_equal)
            # fold 100 shifts -> count (planes are contiguous 100-col runs)
            nc.vector.tensor_tensor(out=meqE[:], in0=meqE[:], in1=meqO[:],
                                    op=ALU.add)                       # 50
            nc.vector.tensor_tensor(out=meqE[:, 0:2500], in0=meqE[:, 0:2500],
                                    in1=meqE[:, 2500:5000], op=ALU.add)  # 25
            nc.vector.tensor_tensor(out=meqE[:, 0:1200], in0=meqE[:, 0:1200],
                                    in1=meqE[:, 1200:2400], op=ALU.add)  # 12
            nc.vector.tensor_tensor(out=meqE[:, 0:600], in0=meqE[:, 0:600],
                                    in1=meqE[:, 600:1200], op=ALU.add)   # 6
            nc.vector.tensor_tensor(out=meqE[:, 0:300], in0=meqE[:, 0:300],
                                    in1=meqE[:, 300:600], op=ALU.add)    # 3
            nc.vector.tensor_tensor(out=meqE[:, 0:100], in0=meqE[:, 0:100],
                                    in1=meqE[:, 200:300], op=ALU.add)    # p0 += p2
            nc.vector.tensor_tensor(out=meqE[:, 100:200], in0=meqE[:, 100:200],
                                    in1=meqE[:, 2400:2500], op=ALU.add)  # p1 += leftover p24
            nc.vector.tensor_tensor(out=cnt16[:N], in0=meqE[:, 0:100],
                                    in1=meqE[:, 100:200], op=ALU.add)

            cmat = cmp_.tile([NCH, L], F16, tag="cmat")
            nc.gpsimd.local_scatter(out_ap=cmat[:], data_ap=cnt16[:],
                                    idxs_ap=g_sb[:, b * N:(b + 1) * N],
                                    channels=NCH, num_elems=L, num_idxs=N)
            cmats[b] = cmat

        # ---- PE: MxT = M2^T-panels x xT  (two halves of 4 kt-psums) ----
        mxt_sb = const.tile([128, DT * R], F16)
        for quarter in range(4):
            pts = []
            for _k2 in range(2):
                pt_mxt = psA.tile([128, R], F32, tag="mxt")
                pts.append(pt_mxt)
            for mt in range(DT):
                for k2 in range(2):
                    kt = quarter * 2 + k2
                    nc.tensor.matmul(
                        out=pts[k2][:],
                        lhsT=m2_sb[:, mt * D + kt * 128:mt * D + (kt + 1) * 128],
                        rhs=xt_sb[:, mt * R:(mt + 1) * R],
                        start=(mt == 0), stop=(mt == DT - 1))
            for k2 in range(2):
                kt = quarter * 2 + k2
                nc.scalar.activation(out=mxt_sb[:, kt * R:(kt + 1) * R],
                                     in_=pts[k2][:], func=ACT.Copy)

        # ---- P logits + relu -> alpha ; M2T -> softmax -> smT ----
        smts = []
        for b in range(BPC):
            bs = slice(b * N, (b + 1) * N)
            pb = psB.tile([N, N], F32, tag="pp")
            for mt in range(DT):
                nc.tensor.matmul(
                    out=pb[:],
                    lhsT=xt_sb[:, mt * R + b * N:mt * R + (b + 1) * N],
                    rhs=mxt_sb[:, mt * R + b * N:mt * R + (b + 1) * N],
                    start=(mt == 0), stop=False)
            nc.tensor.matmul(out=pb[:], lhsT=auglhs[:, bs],
                             rhs=augrhs[:, bs], start=False, stop=True)
            alpha = small.tile([N, N], F16, tag=f"alpha{b}")
            nc.scalar.activation(out=alpha[:], in_=pb[:], func=ACT.Relu)

            pm = psB.tile([N, N], F32, tag="pp")
            nc.tensor.matmul(out=pm[:], lhsT=alpha[:], rhs=adjt_sb[:, bs],
                             start=True, stop=True)
            negmx = small.tile([N, 1], F32, tag=f"ngm{b}")
            nc.vector.tensor_reduce(out=negmx[:], in_=pm[:],
                                    axis=mybir.AxisListType.X,
                                    op=ALU.max, negate=True)
            sm_sb = small.tile([N, N], F32, tag=f"sm{b}")
            ssum = small.tile([N, 1], F32, tag=f"ssum{b}")
            nc.scalar.activation(out=sm_sb[:], in_=pm[:], func=ACT.Exp,
                                 bias=negmx[:], scale=1.0, accum_out=ssum[:])
            rsum = small.tile([N, 1], F32, tag=f"rsum{b}")
            nc.vector.reciprocal(out=rsum[:], in_=ssum[:])
            smt = small.tile([N, N], F16, tag=f"smt{b}")
            nc.scalar.activation(out=smt[:], in_=sm_sb[:], func=ACT.Copy,
                                 scale=rsum[:])
            smts.append(smt)

        # ---- LIN: x@wct accumulation (counts part joins later) ----
        lin_ps = []
        for b in range(BPC):
            lp = pslin.tile([N, D], F32, tag="pslin")
            for dk in range(DT):
                for nch in range(2):
                    sl = slice(nch * 512, (nch + 1) * 512)
                    nc.tensor.matmul(
                        out=lp[:, sl],
                        lhsT=xt_sb[:, dk * R + b * N:dk * R + (b + 1) * N],
                        rhs=wct_sb[:, dk * D + nch * 512:dk * D + nch * 512 + 512],
                        start=(dk == 0), stop=False)
            lin_ps.append(lp)

        # ---- per batch (1 first: its cmat is ready first): C^T
        # transposes, counts matmul, then a 512-column-half pipelined
        # tail: counts-half -> lin evac -> out matmul -> evac -> DMA ----
        for b in (1, 0):
            cmat = cmats[b]
            ct_sb = ctp.tile([128, LT * N], F16, tag="ct")
            for lc in range(LT):
                cs = min(128, L - lc * 128)
                ptt = psB.tile([128, N], F16, tag="pp")
                nc.tensor.transpose(out=ptt[:cs, :],
                                    in_=cmat[:N, lc * 128:lc * 128 + cs],
                                    identity=id16[:N, :N])
                if lc % 2 == 0:
                    nc.scalar.activation(out=ct_sb[:cs, lc * N:(lc + 1) * N],
                                         in_=ptt[:cs, :], func=ACT.Copy)
                else:
                    nc.vector.tensor_copy(out=ct_sb[:cs, lc * N:(lc + 1) * N],
                                          in_=ptt[:cs, :])
            lin_sb = linp.tile([N, D], F16, tag=f"lin{b}")
            po = pslin.tile([N, D], F32, tag="pslin")
            o_sb = outp.tile([N, D], F32, tag="osb")
            for nch in range(2):
                sl = slice(nch * 512, (nch + 1) * 512)
                for lc in range(LT):
                    cs = min(128, L - lc * 128)
                    nc.tensor.matmul(
                        out=lin_ps[b][:, sl],
                        lhsT=ct_sb[:cs, lc * N:(lc + 1) * N],
                        rhs=bias_sb[:cs, lc * D + nch * 512:lc * D + nch * 512 + 512],
                        start=False, stop=(lc == LT - 1))
                nc.vector.tensor_copy(out=lin_sb[:, sl], in_=lin_ps[b][:, sl])
                nc.tensor.matmul(out=po[:, sl], lhsT=smts[b][:],
                                 rhs=lin_sb[:, sl], start=True, stop=True)
                nc.scalar.activation(out=o_sb[:, sl], in_=po[:, sl],
                                     func=ACT.Copy)
                nc.sync.dma_start(
                    out=out_d.ap()[b * N:(b + 1) * N, nch * 512:(nch + 1) * 512],
                    in_=o_sb[:, sl])


def _prep_inputs(feature, graph, W0, W1, bias, dp_Wa, dp_ba, dp_Wb, dp_bb):
    feature = np.asarray(feature, dtype=np.float32)
    graph = np.asarray(graph)
    W0 = np.asarray(W0, np.float32)
    W1 = np.asarray(W1, np.float32)
    bias = np.asarray(bias, np.float32)
    Wa = np.asarray(dp_Wa, np.float32)
    Wb = np.asarray(dp_Wb, np.float32)
    ba = np.asarray(dp_ba, np.float32)
    bb = np.asarray(dp_bb, np.float32)

    M2 = (Wb.T @ Wa).astype(np.float16)               # M^T, M = Wa^T@Wb
    m2r = np.ascontiguousarray(
        M2.reshape(DT, 128, D).transpose(1, 0, 2).reshape(128, DT * D))
    wct = (W0.T + W1.T + np.eye(D, dtype=np.float32)).astype(np.float16)
    wctr = np.ascontiguousarray(
        wct.reshape(DT, 128, D).transpose(1, 0, 2).reshape(128, DT * D))
    bias16 = bias.astype(np.float16)
    biasp = np.zeros((LT * 128, D), np.float16)
    biasp[:L] = bias16
    biasr = np.ascontiguousarray(
        biasp.reshape(LT, 128, D).transpose(1, 0, 2).reshape(128, LT * D))
    id16 = np.eye(128, dtype=np.float16)

    rvec = (feature @ (Wa.T @ bb)).astype(np.float16)     # [B, N]
    cvec = (feature @ (Wb.T @ ba) + ba @ bb).astype(np.float16)

    g16 = graph.astype(np.int16)                          # [B, N, N]
    adj = (graph != 0).astype(np.float16)                 # [B, N, N]

    in_maps = []
    ones = np.ones(N, np.float16)
    for c in range(NCORES):
        bs = slice(c * BPC, (c + 1) * BPC)
        xb = feature[bs].reshape(R, D)
        xt = np.ascontiguousarray(
            xb.T.reshape(DT, 128, R).transpose(1, 0, 2).reshape(128, DT * R)
        ).astype(np.float16)
        gc = g16[bs]                                      # [2, N, N]
        gidx = np.full((NCH, R), -1, np.int16)
        gidx[:N] = gc.transpose(1, 0, 2).reshape(N, R)
        g2 = np.concatenate(
            [gc, gc, np.full((BPC, N, 2), -1, np.int16)], axis=2)  # [2,N,202]
        g2a = np.ascontiguousarray(g2.transpose(1, 0, 2).reshape(N, BPC * G2W))
        g2s = np.roll(g2, -1, axis=2)
        g2b = np.ascontiguousarray(g2s.transpose(1, 0, 2).reshape(N, BPC * G2W))
        adjt = np.ascontiguousarray(
            adj[bs].transpose(2, 0, 1).reshape(N, R))     # [j, b, i]
        auglhs = np.stack([rvec[bs].reshape(R),
                           np.concatenate([ones, ones])])  # [2, R]
        augrhs = np.stack([np.concatenate([ones, ones]),
                           cvec[bs].reshape(R)])
        in_maps.append({
            "gidx": gidx, "g2a": g2a, "g2b": g2b, "id16": id16, "xt": xt,
            "m2": m2r, "auglhs": np.ascontiguousarray(auglhs),
            "augrhs": np.ascontiguousarray(augrhs), "adjt": adjt,
            "wct": wctr, "biasr": biasr,
        })
    return in_maps


def get_program():
    if "nc" not in _CACHE:
        _CACHE["nc"] = _build_program()
    return _CACHE["nc"]


def kernel(feature, graph, W0, W1, bias, dp_Wa, dp_ba, dp_Wb, dp_bb,
           get_alpha=0, **_ignored):
    from concourse.bass_utils import run_bass_kernel_spmd

    nc = get_program()
    in_maps = _prep_inputs(feature, graph, W0, W1, bias, dp_Wa, dp_ba,
                           dp_Wb, dp_bb)
    res = run_bass_kernel_spmd(nc, in_maps, list(range(NCORES)))
    out = np.concatenate(
        [res.results[c]["out"].reshape(BPC, N, D) for c in range(NCORES)],
        axis=0)
    return out


# revision 17
# speedup vs baseline: 1.1542x; 1.1542x over previous
"""Trainium2 Bass kernel for CorrelatedGraphConv (fp16 redesign).

Reference (per batch, N=100 rows, D=1024, L=2000 labels):
    adj   = (graph != 0)
    lin   = x + x@W0.T + x@W1.T + sum_j bias[graph[:, j]]
    a     = x@Wa.T + ba ; b = x@Wb.T + bb
    alpha = relu(a @ b.T)
    alpha = softmax(adj @ alpha, axis=0)     # over rows i
    out   = alpha @ lin

Design (data-parallel, 2 batches/core):
  * QK rewrite: a@b.T = x (Wa.T Wb) x.T + r 1^T + 1 c'^T with
    M2 = Wb.T@Wa (= M^T), r = x@(Wa.T@bb), c' = x@(Wb.T@ba) + ba.bb
    precomputed on host; the rank-1 terms ride an augmented K=2 matmul.
  * wct = W0.T + W1.T + I folds the "+x" into the linear matmul.
  * x^T, adj^T, identity, doubled-g arrays are host-prepared, killing
    all x/g transposes on device.
  * Label histogram: count[i,j] = #(g[i,:]==g[i,j]) via int16 shifted
    compares on DVE (contiguous APs -> 2x mode; even/odd shift split
    keeps 4B alignment), then gpsimd local_scatter writes count at
    idx=g for EVERY token - duplicate indices race benignly because
    all duplicates carry the same value (HW-verified).
  * Everything fp16 (numpy-emulated rel err 9.7e-4 vs 2e-2 budget).
"""

import numpy as np

import concourse.bass as bass
import concourse.mybir as mybir
import concourse.tile as tile
from concourse import bacc, library_config

F32 = mybir.dt.float32
F16 = mybir.dt.float16
I16 = mybir.dt.int16

B, N, D, L = 16, 100, 1024, 2000
NCORES = 8
BPC = B // NCORES          # 2 batches per core
R = BPC * N                # 200 rows per core
DT = D // 128              # 8 d-tiles
LT = (L + 127) // 128      # 16 label tiles (last is 80 rows)
NCH = 112                  # scatter channels (>=100, mult of 16)
G2W = 2 * N + 2            # doubled g row + pad

_CACHE = {}

ACT = mybir.ActivationFunctionType
ALU = mybir.AluOpType


def _ap3(sl, mid_step, mid_cnt, inner_cnt):
    """[P, F] contiguous slice -> [P, mid, inner] view with raw steps."""
    (pstep, pcount), (fstep, fcount) = sl.ap[0], sl.ap[1]
    assert fstep == 1
    return bass.AP(tensor=sl.tensor, offset=sl.offset,
                   ap=[[pstep, pcount], [mid_step, mid_cnt],
                       [1, inner_cnt]])


def _build_program():
    nc = bacc.Bacc("TRN2", target_bir_lowering=False, debug=False,
                   num_devices=NCORES)
    g_d = nc.declare_dram_parameter("gidx", [NCH, R], I16, isOutput=False)
    g2a_d = nc.declare_dram_parameter("g2a", [N, BPC * G2W], I16, isOutput=False)
    g2b_d = nc.declare_dram_parameter("g2b", [N, BPC * G2W], I16, isOutput=False)
    id16_d = nc.declare_dram_parameter("id16", [128, 128], F16, isOutput=False)
    xt_d = nc.declare_dram_parameter("xt", [128, DT * R], F16, isOutput=False)
    m2_d = nc.declare_dram_parameter("m2", [128, DT * D], F16, isOutput=False)
    auglhs_d = nc.declare_dram_parameter("auglhs", [2, R], F16, isOutput=False)
    augrhs_d = nc.declare_dram_parameter("augrhs", [2, R], F16, isOutput=False)
    adjt_d = nc.declare_dram_parameter("adjt", [N, R], F16, isOutput=False)
    wct_d = nc.declare_dram_parameter("wct", [128, DT * D], F16, isOutput=False)
    bias_d = nc.declare_dram_parameter("biasr", [128, LT * D], F16, isOutput=False)
    out_d = nc.declare_dram_parameter("out", [R, D], F32, isOutput=True)

    with tile.TileContext(nc) as tc:
        _emit(tc, g_d, g2a_d, g2b_d, id16_d, xt_d, m2_d, auglhs_d,
              augrhs_d, adjt_d, wct_d, bias_d, out_d)
    nc.compile()
    return nc


def _emit(tc, g_d, g2a_d, g2b_d, id16_d, xt_d, m2_d, auglhs_d, augrhs_d,
          adjt_d, wct_d, bias_d, out_d):
    nc = tc.nc
    import contextlib

    ctx = contextlib.ExitStack()
    with ctx:
        const = ctx.enter_context(tc.tile_pool(name="const", bufs=1))
        meqp = ctx.enter_context(tc.tile_pool(name="meq", bufs=1))
        cbuf = ctx.enter_context(tc.tile_pool(name="cbuf", bufs=4))
        cmp_ = ctx.enter_context(tc.tile_pool(name="cmat", bufs=2))
        ctp = ctx.enter_context(tc.tile_pool(name="ctm", bufs=2))
        small = ctx.enter_context(tc.tile_pool(name="small", bufs=8))
        linp = ctx.enter_context(tc.tile_pool(name="lin", bufs=2))
        outp = ctx.enter_context(tc.tile_pool(name="outs", bufs=2))
        psA = ctx.enter_context(tc.tile_pool(name="psA", bufs=2, space="PSUM"))
        psB = ctx.enter_context(tc.tile_pool(name="psB", bufs=2, space="PSUM"))
        pslin = ctx.enter_context(tc.tile_pool(name="pslin", bufs=2, space="PSUM"))

        nc.gpsimd.load_library(library_config.local_scatter)

        # ---- input DMAs, ordered by first use ----
        g2a = const.tile([N, BPC * G2W], I16)
        nc.sync.dma_start(out=g2a[:], in_=g2a_d.ap())
        g_sb = const.tile([NCH, R], I16)
        nc.sync.dma_start(out=g_sb[:], in_=g_d.ap())
        g2b = const.tile([N, BPC * G2W], I16)
        nc.sync.dma_start(out=g2b[:], in_=g2b_d.ap())
        xt_sb = const.tile([128, DT * R], F16)
        nc.sync.dma_start(out=xt_sb[:], in_=xt_d.ap())
        m2_sb = const.tile([128, DT * D], F16)
        for mt in range(DT):
            nc.sync.dma_start(out=m2_sb[:, mt * D:(mt + 1) * D],
                              in_=m2_d.ap()[:, mt * D:(mt + 1) * D])
        auglhs = const.tile([2, R], F16)
        nc.sync.dma_start(out=auglhs[:], in_=auglhs_d.ap())
        augrhs = const.tile([2, R], F16)
        nc.sync.dma_start(out=augrhs[:], in_=augrhs_d.ap())
        adjt_sb = const.tile([N, R], F16)
        nc.sync.dma_start(out=adjt_sb[:], in_=adjt_d.ap())
        id16 = const.tile([128, 128], F16)
        nc.sync.dma_start(out=id16[:], in_=id16_d.ap())
        wct_sb = const.tile([128, DT * D], F16)
        for dk in range(DT):
            nc.sync.dma_start(out=wct_sb[:, dk * D:(dk + 1) * D],
                              in_=wct_d.ap()[:, dk * D:(dk + 1) * D])
        bias_sb = const.tile([128, LT * D], F16)
        for lc in range(LT):
            nc.sync.dma_start(out=bias_sb[:, lc * D:(lc + 1) * D],
                              in_=bias_d.ap()[:, lc * D:(lc + 1) * D])

        # ---- DVE dedup chain + scatter (meq pool bufs=1 serializes the
        # two chains; interleaving them only delays the first cmat) ----
        cmats = {}
        for b in (0, 1):
            gsl = g_sb[:N, b * N:(b + 1) * N]
            cnt16 = cbuf.tile([NCH, N], F16, tag=f"cnt{b}")

            meqE = meqp.tile([N, 50 * N], I16, tag="meqE")
            meqO = meqp.tile([N, 50 * N], I16, tag="meqO")
            # meqE[i,t,j] = (g[i,j] == g2[i, 2t + j]);  meqO: 2t+1 + j
            nc.vector.tensor_tensor(
                out=_ap3(meqE[:], N, 50, N),
                in0=_ap3(gsl, 0, 50, N),
                in1=_ap3(g2a[:, b * G2W:b * G2W + 2 * N], 2, 50, N),
                op=ALU.is_equal)
            nc.vector.tensor_tensor(
                out=_ap3(meqO[:], N, 50, N),
                in0=_ap3(gsl, 0, 50, N),
                in1=_ap3(g2b[:, b * G2W:b * G2W + 2 * N], 2, 50, N),
                op=ALU.is_equal)
            # fold 100 shifts -> count (planes are contiguous 100-col runs)
            nc.vector.tensor_tensor(out=meqE[:], in0=meqE[:], in1=meqO[:],
                                    op=ALU.add)                       # 50
            nc.vector.tensor_tensor(out=meqE[:, 0:2500], in0=meqE[:, 0:2500],
                                    in1=meqE[:, 2500:5000], op=ALU.add)  # 25
            nc.vector.tensor_tensor(out=meqE[:, 0:1200], in0=meqE[:, 0:1200],
                                    in1=meqE[:, 1200:2400], op=ALU.add)  # 12
            nc.vector.tensor_tensor(out=meqE[:, 0:600], in0=meqE[:, 0:600],
                                    in1=meqE[:, 600:1200], op=ALU.add)   # 6
            nc.vector.tensor_tensor(out=meqE[:, 0:300], in0=meqE[:, 0:300],
                                    in1=meqE[:, 300:600], op=ALU.add)    # 3
            nc.vector.tensor_tensor(out=meqE[:, 0:100], in0=meqE[:, 0:100],
                                    in1=meqE[:, 200:300], op=ALU.add)    # p0 += p2
            nc.vector.tensor_tensor(out=meqE[:, 100:200], in0=meqE[:, 100:200],
                                    in1=meqE[:, 2400:2500], op=ALU.add)  # p1 += leftover p24
            nc.vector.tensor_tensor(out=cnt16[:N], in0=meqE[:, 0:100],
                                    in1=meqE[:, 100:200], op=ALU.add)

            cmat = cmp_.tile([NCH, L], F16, tag="cmat")
            nc.gpsimd.local_scatter(out_ap=cmat[:], data_ap=cnt16[:],
                                    idxs_ap=g_sb[:, b * N:(b + 1) * N],
                                    channels=NCH, num_elems=L, num_idxs=N)
            cmats[b] = cmat

        # ---- PE: MxT = M2^T-panels x xT  (two halves of 4 kt-psums) ----
        mxt_sb = const.tile([128, DT * R], F16)
        for quarter in range(4):
            pts = []
            for _k2 in range(2):
                pt_mxt = psA.tile([128, R], F32, tag="mxt")
                pts.append(pt_mxt)
            for mt in range(DT):
                for k2 in range(2):
                    kt = quarter * 2 + k2
                    nc.tensor.matmul(
                        out=pts[k2][:],
                        lhsT=m2_sb[:, mt * D + kt * 128:mt * D + (kt + 1) * 128],
                        rhs=xt_sb[:, mt * R:(mt + 1) * R],
                        start=(mt == 0), stop=(mt == DT - 1))
            for k2 in range(2):
                kt = quarter * 2 + k2
                nc.scalar.activation(out=mxt_sb[:, kt * R:(kt + 1) * R],
                                     in_=pts[k2][:], func=ACT.Copy)

        # ---- P logits + relu -> alpha ; M2T -> softmax -> smT ----
        smts = []
        for b in range(BPC):
            bs = slice(b * N, (b + 1) * N)
            pb = psB.tile([N, N], F32, tag="pp")
            for mt in range(DT):
                nc.tensor.matmul(
                    out=pb[:],
                    lhsT=xt_sb[:, mt * R + b * N:mt * R + (b + 1) * N],
                    rhs=mxt_sb[:, mt * R + b * N:mt * R + (b + 1) * N],
                    start=(mt == 0), stop=False)
            nc.tensor.matmul(out=pb[:], lhsT=auglhs[:, bs],
                             rhs=augrhs[:, bs], start=False, stop=True)
            alpha = small.tile([N, N], F16, tag=f"alpha{b}")
            nc.scalar.activation(out=alpha[:], in_=pb[:], func=ACT.Relu)

            pm = psB.tile([N, N], F32, tag="pp")
            nc.tensor.matmul(out=pm[:], lhsT=alpha[:], rhs=adjt_sb[:, bs],
                             start=True, stop=True)
            negmx = small.tile([N, 1], F32, tag=f"ngm{b}")
            nc.vector.tensor_reduce(out=negmx[:], in_=pm[:],
                                    axis=mybir.AxisListType.X,
                                    op=ALU.max, negate=True)
            sm_sb = small.tile([N, N], F32, tag=f"sm{b}")
            ssum = small.tile([N, 1], F32, tag=f"ssum{b}")
            nc.scalar.activation(out=sm_sb[:], in_=pm[:], func=ACT.Exp,
                                 bias=negmx[:], scale=1.0, accum_out=ssum[:])
            rsum = small.tile([N, 1], F32, tag=f"rsum{b}")
            nc.vector.reciprocal(out=rsum[:], in_=ssum[:])
            smt = small.tile([N, N], F16, tag=f"smt{b}")
            nc.scalar.activation(out=smt[:], in_=sm_sb[:], func=ACT.Copy,
                                 scale=rsum[:])
            smts.append(smt)

        # ---- LIN: x@wct accumulation (counts part joins later) ----
        lin_ps = []
        for b in range(BPC):
            lp = pslin.tile([N, D], F32, tag="pslin")
            for dk in range(DT):
                for nch in range(2):
                    sl = slice(nch * 512, (nch + 1) * 512)
                    nc.tensor.matmul(
                        out=lp[:, sl],
                        lhsT=xt_sb[:, dk * R + b * N:dk * R + (b + 1) * N],
                        rhs=wct_sb[:, dk * D + nch * 512:dk * D + nch * 512 + 512],
                        start=(dk == 0), stop=False)
            lin_ps.append(lp)

        # ---- per batch (1 first: its cmat is ready first): C^T
        # transposes, counts matmul, then a 512-column-half pipelined
        # tail: counts-half -> lin evac -> out matmul -> evac -> DMA ----
        for b in (0, 1):
            cmat = cmats[b]
            ct_sb = ctp.tile([128, LT * N], F16, tag="ct")
            for lc in range(LT):
                cs = min(128, L - lc * 128)
                ptt = psB.tile([128, N], F16, tag="pp")
                nc.tensor.transpose(out=ptt[:cs, :],
                                    in_=cmat[:N, lc * 128:lc * 128 + cs],
                                    identity=id16[:N, :N])
                if lc % 2 == 0:
                    nc.scalar.activation(out=ct_sb[:cs, lc * N:(lc + 1) * N],
                                         in_=ptt[:cs, :], func=ACT.Copy)
                else:
                    nc.vector.tensor_copy(out=ct_sb[:cs, lc * N:(lc + 1) * N],
                                          in_=ptt[:cs, :])
            lin_sb = linp.tile([N, D], F16, tag=f"lin{b}")
            po = pslin.tile([N, D], F32, tag="pslin")
            o_sb = outp.tile([N, D], F32, tag="osb")
            for nch in range(2):
                sl = slice(nch * 512, (nch + 1) * 512)
                for lc in range(LT):
                    cs = min(128, L - lc * 128)
                    nc.tensor.matmul(
                        out=lin_ps[b][:, sl],
                        lhsT=ct_sb[:cs, lc * N:(lc + 1) * N],
                        rhs=bias_sb[:cs, lc * D + nch * 512:lc * D + nch * 512 + 512],
                        start=False, stop=(lc == LT - 1))
                nc.vector.tensor_copy(out=lin_sb[:, sl], in_=lin_ps[b][:, sl])
                nc.tensor.matmul(out=po[:, sl], lhsT=smts[b][:],
                                 rhs=lin_sb[:, sl], start=True, stop=True)
                nc.scalar.activation(out=o_sb[:, sl], in_=po[:, sl],
                                     func=ACT.Copy)
                nc.sync.dma_start(
                    out=out_d.ap()[b * N:(b + 1) * N, nch * 512:(nch + 1) * 512],
                    in_=o_sb[:, sl])


def _prep_inputs(feature, graph, W0, W1, bias, dp_Wa, dp_ba, dp_Wb, dp_bb):
    feature = np.asarray(feature, dtype=np.float32)
    graph = np.asarray(graph)
    W0 = np.asarray(W0, np.float32)
    W1 = np.asarray(W1, np.float32)
    bias = np.asarray(bias, np.float32)
    Wa = np.asarray(dp_Wa, np.float32)
    Wb = np.asarray(dp_Wb, np.float32)
    ba = np.asarray(dp_ba, np.float32)
    bb = np.asarray(dp_bb, np.float32)

    M2 = (Wb.T @ Wa).astype(np.float16)               # M^T, M = Wa^T@Wb
    m2r = np.ascontiguousarray(
        M2.reshape(DT, 128, D).transpose(1, 0, 2).reshape(128, DT * D))
    wct = (W0.T + W1.T + np.eye(D, dtype=np.float32)).astype(np.float16)
    wctr = np.ascontiguousarray(
        wct.reshape(DT, 128, D).transpose(1, 0, 2).reshape(128, DT * D))
    bias16 = bias.astype(np.float16)
    biasp = np.zeros((LT * 128, D), np.float16)
    biasp[:L] = bias16
    biasr = np.ascontiguousarray(
        biasp.reshape(LT, 128, D).transpose(1, 0, 2).reshape(128, LT * D))
    id16 = np.eye(128, dtype=np.float16)

    rvec = (feature @ (Wa.T @ bb)).astype(np.float16)     # [B, N]
    cvec = (feature @ (Wb.T @ ba) + ba @ bb).astype(np.float16)

    g16 = graph.astype(np.int16)                          # [B, N, N]
    adj = (graph != 0).astype(np.float16)                 # [B, N, N]

    in_maps = []
    ones = np.ones(N, np.float16)
    for c in range(NCORES):
        bs = slice(c * BPC, (c + 1) * BPC)
        xb = feature[bs].reshape(R, D)
        xt = np.ascontiguousarray(
            xb.T.reshape(DT, 128, R).transpose(1, 0, 2).reshape(128, DT * R)
        ).astype(np.float16)
        gc = g16[bs]                                      # [2, N, N]
        gidx = np.full((NCH, R), -1, np.int16)
        gidx[:N] = gc.transpose(1, 0, 2).reshape(N, R)
        g2 = np.concatenate(
            [gc, gc, np.full((BPC, N, 2), -1, np.int16)], axis=2)  # [2,N,202]
        g2a = np.ascontiguousarray(g2.transpose(1, 0, 2).reshape(N, BPC * G2W))
        g2s = np.roll(g2, -1, axis=2)
        g2b = np.ascontiguousarray(g2s.transpose(1, 0, 2).reshape(N, BPC * G2W))
        adjt = np.ascontiguousarray(
            adj[bs].transpose(2, 0, 1).reshape(N, R))     # [j, b, i]
        auglhs = np.stack([rvec[bs].reshape(R),
                           np.concatenate([ones, ones])])  # [2, R]
        augrhs = np.stack([np.concatenate([ones, ones]),
                           cvec[bs].reshape(R)])
        in_maps.append({
            "gidx": gidx, "g2a": g2a, "g2b": g2b, "id16": id16, "xt": xt,
            "m2": m2r, "auglhs": np.ascontiguousarray(auglhs),
            "augrhs": np.ascontiguousarray(augrhs), "adjt": adjt,
            "wct": wctr, "biasr": biasr,
        })
    return in_maps


def get_program():
    if "nc" not in _CACHE:
        _CACHE["nc"] = _build_program()
    return _CACHE["nc"]


def kernel(feature, graph, W0, W1, bias, dp_Wa, dp_ba, dp_Wb, dp_bb,
           get_alpha=0, **_ignored):
    from concourse.bass_utils import run_bass_kernel_spmd

    nc = get_program()
    in_maps = _prep_inputs(feature, graph, W0, W1, bias, dp_Wa, dp_ba,
                           dp_Wb, dp_bb)
    res = run_bass_kernel_spmd(nc, in_maps, list(range(NCORES)))
    out = np.concatenate(
        [res.results[c]["out"].reshape(BPC, N, D) for c in range(NCORES)],
        axis=0)
    return out


# revision 31
# speedup vs baseline: 1.1913x; 1.0321x over previous
"""Trainium2 Bass kernel for CorrelatedGraphConv (fp16 redesign).

Reference (per batch, N=100 rows, D=1024, L=2000 labels):
    adj   = (graph != 0)
    lin   = x + x@W0.T + x@W1.T + sum_j bias[graph[:, j]]
    a     = x@Wa.T + ba ; b = x@Wb.T + bb
    alpha = relu(a @ b.T)
    alpha = softmax(adj @ alpha, axis=0)     # over rows i
    out   = alpha @ lin

Design (data-parallel, 2 batches/core):
  * QK rewrite: a@b.T = x (Wa.T Wb) x.T + r 1^T + 1 c'^T with
    M2 = Wb.T@Wa (= M^T), r = x@(Wa.T@bb), c' = x@(Wb.T@ba) + ba.bb
    precomputed on host; the rank-1 terms ride an augmented K=2 matmul.
  * wct = W0.T + W1.T + I folds the "+x" into the linear matmul.
  * x^T, adj^T, identity, doubled-g arrays are host-prepared, killing
    all x/g transposes on device.
  * Label histogram: count[i,j] = #(g[i,:]==g[i,j]) via int16 shifted
    compares on DVE (contiguous APs -> 2x mode; even/odd shift split
    keeps 4B alignment), then gpsimd local_scatter writes count at
    idx=g for EVERY token - duplicate indices race benignly because
    all duplicates carry the same value (HW-verified).
  * Everything fp16 (numpy-emulated rel err 9.7e-4 vs 2e-2 budget).
"""

import numpy as np

import concourse.bass as bass
import concourse.mybir as mybir
import concourse.tile as tile
from concourse import bacc, library_config

F32 = mybir.dt.float32
F16 = mybir.dt.float16
I16 = mybir.dt.int16

B, N, D, L = 16, 100, 1024, 2000
NCORES = 8
BPC = B // NCORES          # 2 batches per core
R = BPC * N                # 200 rows per core
DT = D // 128              # 8 d-tiles
LT = (L + 127) // 128      # 16 label tiles (last is 80 rows)
NCH = 112                  # scatter channels (>=100, mult of 16)
G2W = 2 * N + 2            # doubled g row + pad

_CACHE = {}

ACT = mybir.ActivationFunctionType
ALU = mybir.AluOpType


def _ap3(sl, mid_step, mid_cnt, inner_cnt):
    """[P, F] contiguous slice -> [P, mid, inner] view with raw steps."""
    (pstep, pcount), (fstep, fcount) = sl.ap[0], sl.ap[1]
    assert fstep == 1
    return bass.AP(tensor=sl.tensor, offset=sl.offset,
                   ap=[[pstep, pcount], [mid_step, mid_cnt],
                       [1, inner_cnt]])


def _build_program():
    nc = bacc.Bacc("TRN2", target_bir_lowering=False, debug=False,
                   num_devices=NCORES)
    # gpack packs [gidx | g2a | g2b] so one DMA/semaphore gates the eq chain
    gp_d = nc.declare_dram_parameter("gpack", [NCH, R + 2 * BPC * G2W], I16,
                                     isOutput=False)
    id16_d = nc.declare_dram_parameter("id16", [128, 128], F16, isOutput=False)
    xt_d = nc.declare_dram_parameter("xt", [128, DT * R], F16, isOutput=False)
    m2_d = nc.declare_dram_parameter("m2", [128, DT * D], F16, isOutput=False)
    auglhs_d = nc.declare_dram_parameter("auglhs", [2, R], F16, isOutput=False)
    augrhs_d = nc.declare_dram_parameter("augrhs", [2, R], F16, isOutput=False)
    adjt_d = nc.declare_dram_parameter("adjt", [N, R], F16, isOutput=False)
    wct_d = nc.declare_dram_parameter("wct", [128, DT * D], F16, isOutput=False)
    bias_d = nc.declare_dram_parameter("biasr", [128, LT * D], F16, isOutput=False)
    out_d = nc.declare_dram_parameter("out", [R, D], F32, isOutput=True)

    with tile.TileContext(nc) as tc:
        _emit(tc, gp_d, id16_d, xt_d, m2_d, auglhs_d,
              augrhs_d, adjt_d, wct_d, bias_d, out_d)
    nc.compile()
    return nc


def _emit(tc, gp_d, id16_d, xt_d, m2_d, auglhs_d, augrhs_d,
          adjt_d, wct_d, bias_d, out_d):
    nc = tc.nc
    import contextlib

    ctx = contextlib.ExitStack()
    with ctx:
        const = ctx.enter_context(tc.tile_pool(name="const", bufs=1))
        meqp = ctx.enter_context(tc.tile_pool(name="meq", bufs=1))
        cbuf = ctx.enter_context(tc.tile_pool(name="cbuf", bufs=4))
        cmp_ = ctx.enter_context(tc.tile_pool(name="cmat", bufs=2))
        ctp = ctx.enter_context(tc.tile_pool(name="ctm", bufs=2))
        small = ctx.enter_context(tc.tile_pool(name="small", bufs=8))
        linp = ctx.enter_context(tc.tile_pool(name="lin", bufs=2))
        outp = ctx.enter_context(tc.tile_pool(name="outs", bufs=2))
        psA = ctx.enter_context(tc.tile_pool(name="psA", bufs=2, space="PSUM"))
        psB = ctx.enter_context(tc.tile_pool(name="psB", bufs=2, space="PSUM"))
        pslin = ctx.enter_context(tc.tile_pool(name="pslin", bufs=2, space="PSUM"))

        nc.gpsimd.load_library(library_config.local_scatter)

        # ---- input DMAs, ordered by first use ----
        gpk = const.tile([NCH, R + 2 * BPC * G2W], I16)
        nc.sync.dma_start(out=gpk[:], in_=gp_d.ap())
        xt_sb = const.tile([128, DT * R], F16)
        nc.sync.dma_start(out=xt_sb[:], in_=xt_d.ap())
        m2_sb = const.tile([128, DT * D], F16)
        for mt in range(DT):
            nc.sync.dma_start(out=m2_sb[:, mt * D:(mt + 1) * D],
                              in_=m2_d.ap()[:, mt * D:(mt + 1) * D])
        auglhs = const.tile([2, R], F16)
        nc.sync.dma_start(out=auglhs[:], in_=auglhs_d.ap())
        augrhs = const.tile([2, R], F16)
        nc.sync.dma_start(out=augrhs[:], in_=augrhs_d.ap())
        adjt_sb = const.tile([N, R], F16)
        nc.sync.dma_start(out=adjt_sb[:], in_=adjt_d.ap())
        id16 = const.tile([128, 128], F16)
        nc.sync.dma_start(out=id16[:], in_=id16_d.ap())
        wct_sb = const.tile([128, DT * D], F16)
        for dk in range(DT):
            nc.sync.dma_start(out=wct_sb[:, dk * D:(dk + 1) * D],
                              in_=wct_d.ap()[:, dk * D:(dk + 1) * D])
        bias_sb = const.tile([128, LT * D], F16)
        for lc in range(LT):
            nc.sync.dma_start(out=bias_sb[:, lc * D:(lc + 1) * D],
                              in_=bias_d.ap()[:, lc * D:(lc + 1) * D])

        # ---- DVE dedup chain + scatter (meq pool bufs=1 serializes the
        # two chains; interleaving them only delays the first cmat) ----
        cmats = {}
        for b in (0, 1):
            gsl = gpk[:N, b * N:(b + 1) * N]
            g2a_sl = gpk[:N, R + b * G2W:R + b * G2W + 2 * N]
            g2b_sl = gpk[:N, R + BPC * G2W + b * G2W:
                          R + BPC * G2W + b * G2W + 2 * N]
            cnt16 = cbuf.tile([NCH, N], F16, tag=f"cnt{b}")

            meqE = meqp.tile([N, 50 * N], I16, tag="meqE")
            meqO = meqp.tile([N, 50 * N], I16, tag="meqO")
            # meqE[i,t,j] = (g[i,j] == g2[i, 2t + j]);  meqO: 2t+1 + j
            nc.vector.tensor_tensor(
                out=_ap3(meqE[:], N, 50, N),
                in0=_ap3(gsl, 0, 50, N),
                in1=_ap3(g2a_sl, 2, 50, N),
                op=ALU.is_equal)
            nc.vector.tensor_tensor(
                out=_ap3(meqO[:], N, 50, N),
                in0=_ap3(gsl, 0, 50, N),
                in1=_ap3(g2b_sl, 2, 50, N),
                op=ALU.is_equal)
            # fold 100 shifts -> count (planes are contiguous 100-col runs)
            nc.vector.tensor_tensor(out=meqE[:], in0=meqE[:], in1=meqO[:],
                                    op=ALU.add)                       # 50
            nc.vector.tensor_tensor(out=meqE[:, 0:2500], in0=meqE[:, 0:2500],
                                    in1=meqE[:, 2500:5000], op=ALU.add)  # 25
            nc.vector.tensor_tensor(out=meqE[:, 0:1200], in0=meqE[:, 0:1200],
                                    in1=meqE[:, 1200:2400], op=ALU.add)  # 12
            nc.vector.tensor_tensor(out=meqE[:, 0:600], in0=meqE[:, 0:600],
                                    in1=meqE[:, 600:1200], op=ALU.add)   # 6
            nc.vector.tensor_tensor(out=meqE[:, 0:300], in0=meqE[:, 0:300],
                                    in1=meqE[:, 300:600], op=ALU.add)    # 3
            nc.vector.tensor_tensor(out=meqE[:, 0:100], in0=meqE[:, 0:100],
                                    in1=meqE[:, 200:300], op=ALU.add)    # p0 += p2
            nc.vector.tensor_tensor(out=meqE[:, 100:200], in0=meqE[:, 100:200],
                                    in1=meqE[:, 2400:2500], op=ALU.add)  # p1 += leftover p24
            nc.vector.tensor_tensor(out=cnt16[:N], in0=meqE[:, 0:100],
                                    in1=meqE[:, 100:200], op=ALU.add)

            cmat = cmp_.tile([NCH, L], F16, tag="cmat")
            nc.gpsimd.local_scatter(out_ap=cmat[:], data_ap=cnt16[:],
                                    idxs_ap=gpk[:, b * N:(b + 1) * N],
                                    channels=NCH, num_elems=L, num_idxs=N)
            cmats[b] = cmat

        # ---- PE: MxT = M2^T-panels x xT  (two halves of 4 kt-psums) ----
        mxt_sb = const.tile([128, DT * R], F16)
        for quarter in range(4):
            pts = []
            for _k2 in range(2):
                pt_mxt = psA.tile([128, R], F32, tag="mxt")
                pts.append(pt_mxt)
            for mt in range(DT):
                for k2 in range(2):
                    kt = quarter * 2 + k2
                    nc.tensor.matmul(
                        out=pts[k2][:],
                        lhsT=m2_sb[:, mt * D + kt * 128:mt * D + (kt + 1) * 128],
                        rhs=xt_sb[:, mt * R:(mt + 1) * R],
                        start=(mt == 0), stop=(mt == DT - 1))
            for k2 in range(2):
                kt = quarter * 2 + k2
                nc.scalar.activation(out=mxt_sb[:, kt * R:(kt + 1) * R],
                                     in_=pts[k2][:], func=ACT.Copy)

        # ---- P logits + relu -> alpha ; M2T -> softmax -> smT ----
        smts = []
        for b in range(BPC):
            bs = slice(b * N, (b + 1) * N)
            pb = psB.tile([N, N], F32, tag="pp")
            for mt in range(DT):
                nc.tensor.matmul(
                    out=pb[:],
                    lhsT=xt_sb[:, mt * R + b * N:mt * R + (b + 1) * N],
                    rhs=mxt_sb[:, mt * R + b * N:mt * R + (b + 1) * N],
                    start=(mt == 0), stop=False)
            nc.tensor.matmul(out=pb[:], lhsT=auglhs[:, bs],
                             rhs=augrhs[:, bs], start=False, stop=True)
            alpha = small.tile([N, N], F16, tag=f"alpha{b}")
            nc.scalar.activation(out=alpha[:], in_=pb[:], func=ACT.Relu)

            pm = psB.tile([N, N], F32, tag="pp")
            nc.tensor.matmul(out=pm[:], lhsT=alpha[:], rhs=adjt_sb[:, bs],
                             start=True, stop=True)
            negmx = small.tile([N, 1], F32, tag=f"ngm{b}")
            nc.vector.tensor_reduce(out=negmx[:], in_=pm[:],
                                    axis=mybir.AxisListType.X,
                                    op=ALU.max, negate=True)
            sm_sb = small.tile([N, N], F32, tag=f"sm{b}")
            ssum = small.tile([N, 1], F32, tag=f"ssum{b}")
            nc.scalar.activation(out=sm_sb[:], in_=pm[:], func=ACT.Exp,
                                 bias=negmx[:], scale=1.0, accum_out=ssum[:])
            rsum = small.tile([N, 1], F32, tag=f"rsum{b}")
            nc.vector.reciprocal(out=rsum[:], in_=ssum[:])
            smt = small.tile([N, N], F16, tag=f"smt{b}")
            nc.scalar.activation(out=smt[:], in_=sm_sb[:], func=ACT.Copy,
                                 scale=rsum[:])
            smts.append(smt)

        # ---- LIN: x@wct accumulation (counts part joins later) ----
        lin_ps = []
        for b in range(BPC):
            lp = pslin.tile([N, D], F32, tag="pslin")
            for dk in range(DT):
                for q in range(2):
                    sl = slice(q * 512, (q + 1) * 512)
                    nc.tensor.matmul(
                        out=lp[:, sl],
                        lhsT=xt_sb[:, dk * R + b * N:dk * R + (b + 1) * N],
                        rhs=wct_sb[:, dk * D + q * 512:dk * D + q * 512 + 512],
                        start=(dk == 0), stop=False)
            lin_ps.append(lp)

        # ---- per batch (1 first: its cmat is ready first): C^T
        # transposes, counts matmul, then a 512-column-half pipelined
        # tail: counts-half -> lin evac -> out matmul -> evac -> DMA ----
        for b in (0, 1):
            cmat = cmats[b]
            ct_sb = ctp.tile([128, LT * N], F16, tag="ct")
            for lc in range(LT):
                cs = min(128, L - lc * 128)
                ptt = psB.tile([128, N], F16, tag="pp")
                nc.tensor.transpose(out=ptt[:cs, :],
                                    in_=cmat[:N, lc * 128:lc * 128 + cs],
                                    identity=id16[:N, :N])
                if lc % 2 == 0:
                    nc.scalar.activation(out=ct_sb[:cs, lc * N:(lc + 1) * N],
                                         in_=ptt[:cs, :], func=ACT.Copy)
                else:
                    nc.vector.tensor_copy(out=ct_sb[:cs, lc * N:(lc + 1) * N],
                                          in_=ptt[:cs, :])
            lin_sb = linp.tile([N, D], F16, tag=f"lin{b}")
            po = pslin.tile([N, D], F32, tag="pslin")
            o_sb = outp.tile([N, D], F32, tag="osb")
            for q in range(2):
                sl = slice(q * 512, (q + 1) * 512)
                for lc in range(LT):
                    cs = min(128, L - lc * 128)
                    nc.tensor.matmul(
                        out=lin_ps[b][:, sl],
                        lhsT=ct_sb[:cs, lc * N:(lc + 1) * N],
                        rhs=bias_sb[:cs, lc * D + q * 512:lc * D + q * 512 + 512],
                        start=False, stop=(lc == LT - 1))
                nc.vector.tensor_copy(out=lin_sb[:, sl], in_=lin_ps[b][:, sl])
                nc.tensor.matmul(out=po[:, sl], lhsT=smts[b][:],
                                 rhs=lin_sb[:, sl], start=True, stop=True)
                nc.scalar.activation(out=o_sb[:, sl], in_=po[:, sl],
                                     func=ACT.Copy)
                nc.sync.dma_start(
                    out=out_d.ap()[b * N:(b + 1) * N, q * 512:(q + 1) * 512],
                    in_=o_sb[:, sl])


def _prep_inputs(feature, graph, W0, W1, bias, dp_Wa, dp_ba, dp_Wb, dp_bb):
    feature = np.asarray(feature, dtype=np.float32)
    graph = np.asarray(graph)
    W0 = np.asarray(W0, np.float32)
    W1 = np.asarray(W1, np.float32)
    bias = np.asarray(bias, np.float32)
    Wa = np.asarray(dp_Wa, np.float32)
    Wb = np.asarray(dp_Wb, np.float32)
    ba = np.asarray(dp_ba, np.float32)
    bb = np.asarray(dp_bb, np.float32)

    M2 = (Wb.T @ Wa).astype(np.float16)               # M^T, M = Wa^T@Wb
    m2r = np.ascontiguousarray(
        M2.reshape(DT, 128, D).transpose(1, 0, 2).reshape(128, DT * D))
    wct = (W0.T + W1.T + np.eye(D, dtype=np.float32)).astype(np.float16)
    wctr = np.ascontiguousarray(
        wct.reshape(DT, 128, D).transpose(1, 0, 2).reshape(128, DT * D))
    bias16 = bias.astype(np.float16)
    biasp = np.zeros((LT * 128, D), np.float16)
    biasp[:L] = bias16
    biasr = np.ascontiguousarray(
        biasp.reshape(LT, 128, D).transpose(1, 0, 2).reshape(128, LT * D))
    id16 = np.eye(128, dtype=np.float16)

    rvec = (feature @ (Wa.T @ bb)).astype(np.float16)     # [B, N]
    cvec = (feature @ (Wb.T @ ba) + ba @ bb).astype(np.float16)

    g16 = graph.astype(np.int16)                          # [B, N, N]
    adj = (graph != 0).astype(np.float16)                 # [B, N, N]

    in_maps = []
    ones = np.ones(N, np.float16)
    for c in range(NCORES):
        bs = slice(c * BPC, (c + 1) * BPC)
        xb = feature[bs].reshape(R, D)
        xt = np.ascontiguousarray(
            xb.T.reshape(DT, 128, R).transpose(1, 0, 2).reshape(128, DT * R)
        ).astype(np.float16)
        gc = g16[bs]                                      # [2, N, N]
        gpack = np.full((NCH, R + 2 * BPC * G2W), -1, np.int16)
        gpack[:N, 0:R] = gc.transpose(1, 0, 2).reshape(N, R)
        g2 = np.concatenate(
            [gc, gc, np.full((BPC, N, 2), -1, np.int16)], axis=2)  # [2,N,202]
        gpack[:N, R:R + BPC * G2W] = g2.transpose(1, 0, 2).reshape(N, BPC * G2W)
        g2s = np.roll(g2, -1, axis=2)
        gpack[:N, R + BPC * G2W:] = g2s.transpose(1, 0, 2).reshape(N, BPC * G2W)
        adjt = np.ascontiguousarray(
            adj[bs].transpose(2, 0, 1).reshape(N, R))     # [j, b, i]
        auglhs = np.stack([rvec[bs].reshape(R),
                           np.concatenate([ones, ones])])  # [2, R]
        augrhs = np.stack([np.concatenate([ones, ones]),
                           cvec[bs].reshape(R)])
        in_maps.append({
            "gpack": gpack, "id16": id16, "xt": xt,
            "m2": m2r, "auglhs": np.ascontiguousarray(auglhs),
            "augrhs": np.ascontiguousarray(augrhs), "adjt": adjt,
            "wct": wctr, "biasr": biasr,
        })
    return in_maps


def get_program():
    if "nc" not in _CACHE:
        _CACHE["nc"] = _build_program()
    return _CACHE["nc"]


def kernel(feature, graph, W0, W1, bias, dp_Wa, dp_ba, dp_Wb, dp_bb,
           get_alpha=0, **_ignored):
    from concourse.bass_utils import run_bass_kernel_spmd

    nc = get_program()
    in_maps = _prep_inputs(feature, graph, W0, W1, bias, dp_Wa, dp_ba,
                           dp_Wb, dp_bb)
    res = run_bass_kernel_spmd(nc, in_maps, list(range(NCORES)))
    out = np.concatenate(
        [res.results[c]["out"].reshape(BPC, N, D) for c in range(NCORES)],
        axis=0)
    return out
